# revision 7
# baseline (speedup 1.0000x reference)
"""Trainium2 Bass kernel for ClassCentersEMA (vq_codebook).

Reference semantics (B=16384, D=1024, C=512):
    feats_n   = feats / max(||feats||_row, eps)
    counts    = targets.sum(0)                       # [C]
    class_sums= targets^T @ feats_n                  # [C, D]
    mask      = counts > 0
    means     = class_sums / max(counts, 1)          # rows 0 where !mask
    new       = !initialized & mask
    base      = where(new, means, centers)
    blended   = 0.9*base + 0.1*means
    upd       = where(mask, blended, centers)
    out       = where(mask, upd / max(||upd||, eps), upd)

Distribution: data-parallel over B across 8 cores. Each core computes a
local [C] count and [C, D] class-sum partial; ONE fp16 ReduceScatter
(class rows + a counts row per 65-row slab) hands core r the 64 classes
[64r, 64r+64); the epilogue runs on that shard and the host concatenates
the 8 [64, 1024] outputs.

Matmul precision: plain bf16 (targets are exactly representable; feats
rounding gives ~1e-4 rel err, far inside the 2e-2 gate). counts are
accumulated on DVE in f32 and PE-reduced across partitions once at the
end; fp16 keeps integer counts <= 2048 exact through the collective.

Schedule: a single streaming pass (4 groups of 4 k-tiles, 2 MiB feats +
1 MiB targets HWDGE transfers with a p-major row layout so every
partition reads contiguous HBM). PE consumes tiles as they land into 8
persistent PSUM groups (4 class-blocks x 2 D-halves). After the last
k-tile: PSUM drains cast to fp16 into slabbed bounce buffers and the
single ReduceScatter fires.

Cross-iteration pipelining (what the unrolled slope bench measures):
the epilogue of iteration i is EMITTED in the middle of iteration i+1's
streaming phase, and all post-collective DMAs (cnt/cs loads, out store)
ride the scalar-engine ring. The sync ring therefore carries only
pre-collective traffic and never stalls on the collective semaphore, so
iteration i's ReduceScatter + epilogue overlap iteration i+1's
streaming; the steady-state period approaches the 12 MiB/core HBM
floor.

The epilogue is algebraically folded to
    upd = a_c * centers + b_c * class_sums
with per-class scalars  (a, b):
    !mask:            (1.0, 0)
    mask & inited:    (0.9, 0.1/counts)
    mask & !inited:   (0.0, 1.0/counts)
followed by a masked renormalize.
"""

import numpy as np

import concourse.bass as bass
import concourse.mybir as mybir
import concourse.tile as tile
from concourse import bacc
from concourse.bass_utils import run_bass_kernel_spmd

F32 = mybir.dt.float32
F16 = mybir.dt.float16
BF16 = mybir.dt.bfloat16
AF = mybir.ActivationFunctionType
ALU = mybir.AluOpType

NCORES = 8
B, D, C = 16384, 1024, 512
BL = B // NCORES          # 2048 rows per core
KT = BL // 128            # 16 k-tiles of 128
NG = 4                    # streaming groups
GA = KT // NG             # 4 k-tiles per group
CL = C // NCORES          # 64 classes per core after ReduceScatter
MOM = 0.9
EPS = 1e-12


def build_nc(niters=1):
    """niters>1 unrolls the whole kernel body N times in one NEFF —
    used only for timing (slope over N isolates device exec time)."""
    nc = bacc.Bacc("TRN2", target_bir_lowering=False, debug=False,
                   num_devices=NCORES)

    feats = nc.dram_tensor("feats", [BL, D], F32, kind="ExternalInput")
    targets = nc.dram_tensor("targets", [BL, C], F32, kind="ExternalInput")
    centers = nc.dram_tensor("centers", [CL, D], F32, kind="ExternalInput")
    inited = nc.dram_tensor("inited", [CL, 1], F32, kind="ExternalInput")
    out = nc.dram_tensor("out", [CL, D], F32, kind="ExternalOutput")

    rg = [list(range(NCORES))]

    with tile.TileContext(nc) as tc:
        with (
            tc.tile_pool(name="dram", bufs=2, space="DRAM") as dram,
            tc.tile_pool(name="ftp", bufs=2) as ftp,
            tc.tile_pool(name="tgp", bufs=2) as tgp,
            tc.tile_pool(name="hip", bufs=2) as hip,
            tc.tile_pool(name="sq", bufs=2) as sqp,
            tc.tile_pool(name="small", bufs=8) as small,
            tc.tile_pool(name="single", bufs=1) as single,
            tc.tile_pool(name="cs", bufs=2) as csp,
            tc.tile_pool(name="psum", bufs=8, space="PSUM") as ppool,
            tc.tile_pool(name="epi", bufs=2) as epi,
        ):
            io = dict(feats=feats, targets=targets, centers=centers,
                      inited=inited, out=out)

            ones = single.tile([128, 1], BF16, name="ones")
            nc.vector.memset(ones[:], 1.0)
            eps2 = single.tile([128, 1], F32, name="eps2")
            nc.vector.memset(eps2[:], EPS * EPS)
            zrow = single.tile([1, 1024 - CL], F16, name="zrow")
            nc.vector.memset(zrow[:], 0.0)
            consts = dict(ones=ones, eps2=eps2, zrow=zrow)

            pools = dict(ftp=ftp, tgp=tgp, hip=hip, sqp=sqp, small=small,
                         csp=csp, ppool=ppool, epi=epi, dram=dram)
            prev = None
            for _ in range(niters):
                prev = _emit_iteration(nc, tc, io, consts, pools, rg, prev)
            _emit_epilogue(nc, consts, pools, io, prev)

    nc.compile()
    return nc


def _emit_iteration(nc, tc, io, consts, pools, rg, prev):
    feats, targets = io["feats"], io["targets"]
    centers, inited = io["centers"], io["inited"]
    ones, eps2, zrow = consts["ones"], consts["eps2"], consts["zrow"]
    ftp, tgp, hip, sqp = pools["ftp"], pools["tgp"], pools["hip"], pools["sqp"]
    small, csp, ppool = pools["small"], pools["csp"], pools["ppool"]
    epi, dram = pools["epi"], pools["dram"]

    # bounce buffers (double buffered so iteration i+1's drains never
    # wait on iteration i's collective)
    rs_in = dram.tile([NCORES, CL + 1, D], F16, tag="rs_in", name="rs_in")
    rs_out = dram.tile([CL + 1, D], F16, tag="rs_out", name="rs_out")

    # epilogue inputs that depend on nothing — issue DMAs up front
    ctr = epi.tile([CL, D], F32, tag="ctr", name="ctr")
    nc.sync.dma_start(ctr[:], centers[:])
    ini = epi.tile([CL, 1], F32, tag="ini", name="ini")
    nc.sync.dma_start(ini[:], inited[:])
    # zero the unread tail of every slab's counts row
    for r in range(NCORES):
        nc.sync.dma_start(rs_in[r, CL:CL + 1, CL:D], zrow[:])

    # 8 persistent PSUM accumulators: class block c (128 classes) x
    # D-half h; all live for the whole k loop
    ps = [[ppool.tile([128, 512], F32, tag="acc", name=f"ps{c}{h}")
           for h in range(2)] for c in range(4)]

    # p-major row layout: partition p holds DRAM rows [16p, 16p+16), so
    # each 4-k-tile transfer reads 16 KiB contiguous per partition
    feats_t = feats.rearrange("(p a) d -> p a d", p=128)
    targets_t = targets.rearrange("(p a) c -> p a c", p=128)

    # local per-class counts, accumulated on DVE in f32
    cacc = small.tile([128, C], F32, tag="cacc", bufs=2, name="cacc")
    nc.vector.memset(cacc[:], 0.0)

    for q in range(NG):
        ft = ftp.tile([128, GA, D], F32, tag="ft", name="ft")
        nc.sync.dma_start(ft[:], feats_t[:, GA * q:GA * (q + 1), :])
        tg = tgp.tile([128, GA, C], F32, tag="tg", name="tg")
        nc.sync.dma_start(tg[:], targets_t[:, GA * q:GA * (q + 1), :])

        # counts partial: cacc += sum_a tg[:, a, :]
        t01 = small.tile([128, C], F32, tag="t01", name="t01")
        nc.vector.tensor_add(t01[:], tg[:, 0, :], tg[:, 1, :])
        t23 = small.tile([128, C], F32, tag="t23", name="t23")
        nc.vector.tensor_add(t23[:], tg[:, 2, :], tg[:, 3, :])
        nc.vector.tensor_add(t01[:], t01[:], t23[:])
        nc.vector.tensor_add(cacc[:], cacc[:], t01[:])

        # bf16 cast of targets for the PE (exact: values are 0/1)
        tgb = tgp.tile([128, GA, C], BF16, tag="tgb", name="tgb")
        nc.vector.tensor_copy(tgb[:], tg[:])

        # row norms: Square+accum per sub-tile, one batched sqrt/rcp.
        # sqrt(ssq + eps^2) == max(sqrt(ssq), eps) for any non-degenerate
        # row, and exactly eps for a zero row
        ssq = small.tile([128, GA], F32, tag="ssq", name="ssq")
        for a in range(GA):
            sq = sqp.tile([128, D], F32, tag="sq", name="sq")
            nc.scalar.activation(sq[:], ft[:, a, :], AF.Square,
                                 accum_out=ssq[:, a:a + 1])
        nrm = small.tile([128, GA], F32, tag="nrm", name="nrm")
        nc.scalar.activation(nrm[:], ssq[:], AF.Sqrt, bias=eps2[:])
        rcp = small.tile([128, GA], F32, tag="rcp", name="rcp")
        nc.vector.reciprocal(rcp[:], nrm[:])

        # normalized feats in bf16; alternate DVE/ACT by parity
        hi = hip.tile([128, GA, D], BF16, tag="hi", name="hi")
        for a in range(GA):
            if a % 2 == 0:
                nc.vector.tensor_scalar_mul(hi[:, a, :], ft[:, a, :],
                                            rcp[:, a:a + 1])
            else:
                nc.scalar.activation(hi[:, a, :], ft[:, a, :], AF.Copy,
                                     scale=rcp[:, a:a + 1])

        for a in range(GA):
            k = GA * q + a
            for c in range(4):
                lhs = tgb[:, a, c * 128:(c + 1) * 128]
                for h in range(2):
                    nc.tensor.matmul(ps[c][h][:], lhs,
                                     hi[:, a, 512 * h:512 * (h + 1)],
                                     start=(k == 0), stop=(k == KT - 1))

        # iteration i-1's epilogue drops in here: by the time the
        # scalar/vector queues reach it, that collective has landed
        if q == 2 and prev is not None:
            _emit_epilogue(nc, consts, pools, io, prev)
            prev = None

    # ---- drain + single ReduceScatter ----
    # class block c -> slab rows: global class 128c+j lives in slab
    # 2c + j//64, row j%64; both D-halves packed in one 1024-wide row
    for c in range(4):
        comb = csp.tile([128, D], F16, tag="comb", name="comb")
        nc.vector.tensor_copy(comb[:, 0:512], ps[c][0][:])
        nc.scalar.copy(comb[:, 512:1024], ps[c][1][:])
        nc.sync.dma_start(rs_in[2 * c, 0:CL, :], comb[0:CL, :])
        nc.sync.dma_start(rs_in[2 * c + 1, 0:CL, :], comb[CL:128, :])

    # counts: fold cacc across partitions on the PE, then scatter the
    # per-rank chunks into each slab's row CL
    cacc_bf = small.tile([128, C], BF16, tag="cacc_bf", bufs=2,
                         name="cacc_bf")
    nc.vector.tensor_copy(cacc_bf[:], cacc[:])
    cps = ppool.tile([1, C], F32, tag="acc", name="cps")
    nc.tensor.matmul(cps[:], ones[:], cacc_bf[:], start=True, stop=True)
    cnt16 = small.tile([1, C], F16, tag="cnt16", bufs=2, name="cnt16")
    nc.vector.tensor_copy(cnt16[:], cps[:])
    for r in range(NCORES):
        nc.sync.dma_start(rs_in[r, CL:CL + 1, 0:CL],
                          cnt16[0:1, r * CL:(r + 1) * CL])

    nc.gpsimd.collective_compute(
        "ReduceScatter", ALU.add, replica_groups=rg,
        ins=[rs_in[:].opt()], outs=[rs_out[:].opt()])

    return dict(rs_out=rs_out, ctr=ctr, ini=ini)


def _emit_epilogue(nc, consts, pools, io, prev):
    """Epilogue on this core's CL classes; reads prev['rs_out'].
    All DMAs ride the scalar ring so the sync ring never waits on the
    collective."""
    eps2 = consts["eps2"]
    epi = pools["epi"]
    out = io["out"]
    rs_out, ctr, ini = prev["rs_out"], prev["ctr"], prev["ini"]

    cnt16t = epi.tile([CL, 1], F16, tag="cnt16t", name="cnt16t")
    nc.scalar.dma_start(
        cnt16t[:],
        rs_out[CL:CL + 1, 0:CL].rearrange("a c -> (a c)").unsqueeze(1))
    cnt = epi.tile([CL, 1], F32, tag="cnt", name="cnt")
    nc.vector.tensor_copy(cnt[:], cnt16t[:])
    cs16 = epi.tile([CL, D], F16, tag="cs16", name="cs16")
    nc.scalar.dma_start(cs16[:], rs_out[0:CL, :])
    cs = epi.tile([CL, D], F32, tag="cs", name="cs")
    nc.scalar.copy(cs[:], cs16[:])

    mask = epi.tile([CL, 1], F32, tag="mask", name="mask")
    nc.vector.tensor_scalar_min(mask[:], cnt[:], 1.0)
    omask = epi.tile([CL, 1], F32, tag="omask", name="omask")
    nc.vector.tensor_scalar(omask[:], mask[:], -1.0, 1.0,
                            op0=ALU.mult, op1=ALU.add)
    inv = epi.tile([CL, 1], F32, tag="inv", name="inv")
    nc.vector.tensor_scalar_max(inv[:], cnt[:], 1.0)
    nc.vector.reciprocal(inv[:], inv[:])
    # new01 = (1 - inited) * mask
    new01 = epi.tile([CL, 1], F32, tag="new01", name="new01")
    nc.vector.tensor_scalar(new01[:], ini[:], -1.0, 1.0,
                            op0=ALU.mult, op1=ALU.add)
    nc.vector.tensor_mul(new01[:], new01[:], mask[:])
    # b = mask * (0.1 + 0.9*new01); a = 1 - b; bp = b / max(cnt,1)
    bco = epi.tile([CL, 1], F32, tag="bco", name="bco")
    nc.vector.tensor_scalar(bco[:], new01[:], MOM, 1.0 - MOM,
                            op0=ALU.mult, op1=ALU.add)
    nc.vector.tensor_mul(bco[:], bco[:], mask[:])
    aco = epi.tile([CL, 1], F32, tag="aco", name="aco")
    nc.vector.tensor_scalar(aco[:], bco[:], -1.0, 1.0,
                            op0=ALU.mult, op1=ALU.add)
    nc.vector.tensor_mul(bco[:], bco[:], inv[:])

    upd = epi.tile([CL, D], F32, tag="upd", name="upd")
    nc.vector.tensor_scalar_mul(upd[:], ctr[:], aco[:])
    nc.vector.scalar_tensor_tensor(upd[:], cs[:], bco[:], upd[:],
                                   op0=ALU.mult, op1=ALU.add)

    # masked renormalize
    usq = epi.tile([CL, D], F32, tag="usq", name="usq")
    ussq = epi.tile([CL, 1], F32, tag="ussq", name="ussq")
    nc.scalar.activation(usq[:], upd[:], AF.Square, accum_out=ussq[:])
    unrm = epi.tile([CL, 1], F32, tag="unrm", name="unrm")
    nc.scalar.activation(unrm[:], ussq[:], AF.Sqrt, bias=eps2[0:CL, :])
    urcp = epi.tile([CL, 1], F32, tag="urcp", name="urcp")
    nc.vector.reciprocal(urcp[:], unrm[:])
    # rfin = mask*urcp + (1-mask), one fused tensor_scalar
    nc.vector.tensor_scalar(urcp[:], mask[:], urcp[:], omask[:],
                            op0=ALU.mult, op1=ALU.add)

    ov = epi.tile([CL, D], F32, tag="ov", name="ov")
    nc.vector.tensor_scalar_mul(ov[:], upd[:], urcp[:])
    nc.scalar.dma_start(out[:], ov[:])


_NC_CACHE = None


def _get_nc():
    global _NC_CACHE
    if _NC_CACHE is None:
        _NC_CACHE = build_nc()
    return _NC_CACHE


def run_spmd(feats, targets, centers, initialized, **kw):
    feats = np.ascontiguousarray(np.asarray(feats, dtype=np.float32))
    targets = np.ascontiguousarray(np.asarray(targets, dtype=np.float32))
    centers = np.ascontiguousarray(np.asarray(centers, dtype=np.float32))
    init_f = np.asarray(initialized).astype(np.float32).reshape(C, 1)
    assert feats.shape == (B, D) and targets.shape == (B, C)
    assert centers.shape == (C, D)

    nc = _get_nc()
    in_maps = []
    for r in range(NCORES):
        in_maps.append({
            "feats": feats[r * BL:(r + 1) * BL],
            "targets": targets[r * BL:(r + 1) * BL],
            "centers": np.ascontiguousarray(centers[r * CL:(r + 1) * CL]),
            "inited": np.ascontiguousarray(init_f[r * CL:(r + 1) * CL]),
        })
    res = run_bass_kernel_spmd(nc, in_maps, core_ids=list(range(NCORES)), **kw)
    out = np.concatenate([res.results[r]["out"] for r in range(NCORES)], axis=0)
    return out.astype(np.float32), res


def kernel(feats, targets, centers, initialized):
    out, _ = run_spmd(feats, targets, centers, initialized)
    return out
